# revision 18
# baseline (speedup 1.0000x reference)
"""Multi-head causal attention (B=4, S=2048, D=1024, H=16) on 8 TRN2 NeuronCores.

Sharding: batch x head-group. Core c handles batch c//2 and heads
8*(c%2) .. 8*(c%2)+8 (tensor parallel over heads). Each core computes its
8 heads' attention plus a partial output projection; the host sums the two
partials per batch and adds b_out.

Device pipeline (per core):
  - projections in fp16 (x, weights pre-cast on host), attention scores via
    row-packed (tile_position) fp16 matmuls in S^T [k, q] layout, exp on
    ScalarE straight out of PSUM, fp16 P with multiplicative mask tiles for
    diagonal blocks (block structure derived from the actual mask input;
    fully-masked blocks skipped, leading masked columns excluded from exp).
  - attn@V with lhsT = [v_h | ones] (M=65): row 64 accumulates softmax
    denominators; normalization = reciprocal_approx_fast + GpSimd partition
    broadcast + multiply.
  - projection chains for later head pairs are emitted interleaved into the
    attention event stream so the PE never drains while ScalarE runs exp.
  - partial out-projection [token, d_model] per core; host sums partials.
"""
import numpy as np

import concourse.bass as bass
import concourse.tile as tile
from concourse import bacc, mybir
from concourse import bass_utils

B, S, D, H, HD = 4, 2048, 1024, 16, 64
NCORES = 8
HPC = H // 2          # heads per core (8)
NPAIR = HPC // 2      # head pairs per core (4)
DC = HPC * HD         # attn dims per core (512)
QT = 512              # q tile (free dim of S^T)
KT = 128              # k tile (partition dim of S^T)
NQT = S // QT         # 4
NKT = S // KT         # 16
NTT = S // 128        # 16 token tiles
NCH = D // 128        # 8 d_model chunks
SCALE = HD ** -0.5

F32 = mybir.dt.float32
F32R = mybir.dt.float32r
F16 = mybir.dt.float16

_cache = {}


def _classify_mask(mask):
    """Per (kt, qt) block: 0=skip (all masked), 1=full (none masked), 2=partial."""
    classes = np.zeros((NKT, NQT), np.int8)
    patterns = []
    pat_idx = {}
    pat_key = {}
    bounds = {}
    for qt in range(NQT):
        mb = mask[qt * QT:(qt + 1) * QT, :]          # [512, S] (q, k)
        for kt in range(NKT):
            blk = mb[:, kt * KT:(kt + 1) * KT]       # [512, 128] (q, k)
            if blk.all():
                classes[kt, qt] = 0
            elif not blk.any():
                classes[kt, qt] = 1
            else:
                classes[kt, qt] = 2
                tilev = (~blk).T.astype(np.float32)  # [128, 512] (k, q), 1=keep
                col_masked = tilev.min(axis=0) == 0.0
                col_dead = tilev.max(axis=0) == 0.0
                zlo = 0
                while zlo < QT and col_dead[zlo]:
                    zlo += 1
                nz = np.nonzero(col_masked[zlo:])[0]
                if len(nz):
                    mlo, mhi = zlo + int(nz[0]), zlo + int(nz[-1]) + 1
                else:
                    mlo, mhi = zlo, zlo
                key = tilev.tobytes()
                if key not in pat_key:
                    pat_key[key] = len(patterns)
                    patterns.append(tilev)
                pat_idx[(kt, qt)] = pat_key[key]
                bounds[(kt, qt)] = (zlo, mlo, mhi)
    if not patterns:
        patterns.append(np.ones((KT, QT), np.float32))
    return classes, np.stack(patterns), pat_idx, bounds


def _build(classes, pat_idx, bounds, n_pat):
    nc = bacc.Bacc("TRN2", target_bir_lowering=False, debug=False,
                   num_devices=NCORES)

    xT_d = nc.dram_tensor("xT", [D, S], F16, kind="ExternalInput")
    wqk_d = nc.dram_tensor("wqk", [D, 2 * DC], F16, kind="ExternalInput")
    wv_d = nc.dram_tensor("wv", [D, DC], F16, kind="ExternalInput")
    wo_d = nc.dram_tensor("wo", [DC, D], F16, kind="ExternalInput")
    bqk_d = nc.dram_tensor("bqk", [1, 2 * DC], F16, kind="ExternalInput")
    bv_d = nc.dram_tensor("bv", [1, DC], F16, kind="ExternalInput")
    mpat_d = nc.dram_tensor("mpat", [n_pat, KT, QT], F16, kind="ExternalInput")
    out01_d = nc.dram_tensor("out01", [S, D], F32, kind="ExternalOutput")
    out23_d = nc.dram_tensor("out23", [S, D], F32, kind="ExternalOutput")

    wqk_c = wqk_d.ap().rearrange("(c p) n -> c p n", p=128)
    wv_c = wv_d.ap().rearrange("(c p) n -> c p n", p=128)
    wo_c = wo_d.ap().rearrange("(c p) n -> c p n", p=128)
    xT_c = xT_d.ap().rearrange("(c p) n -> c p n", p=128)

    # qk production order: pair tiles (q_p at index p, k_p at NPAIR+p)
    QK01 = [0, 4, 1, 5]          # q0 k0 q1 k1 — before attention group 0
    QK23 = [2, 6, 3, 7]          # produced inside attention group 0

    with tile.TileContext(nc) as tc:
        with (
            tc.tile_pool(name="persist", bufs=1) as persist,
            tc.tile_pool(name="bigpool", bufs=1) as bigpool,
        ):
            # ---- persistent tiles + all input DMAs --------------------
            qkT = [bigpool.tile([128, S], F16, name=f"qkT{p}", tag="qk",
                                bufs=8) for p in range(2 * NPAIR)]
            vext = [persist.tile([128, HPC, HD + 1], F16, name=f"vx{t}",
                                 tag=f"vx{t}") for t in range(NTT)]
            mpat = [persist.tile([KT, QT], F16, name=f"mp{i}", tag=f"mp{i}")
                    for i in range(n_pat)]
            outTn = [bigpool.tile([128, S], F16, name=f"oTn{p}", tag="big",
                                  bufs=4) for p in range(NPAIR)]
            ones_row = persist.tile([1, QT], F16)
            nc.vector.memset(ones_row, 1.0)
            bqk_sb = persist.tile([1, 2 * DC], F16)
            nc.sync.dma_start(bqk_sb, bqk_d.ap())
            bv_sb = persist.tile([1, DC], F16)
            nc.sync.dma_start(bv_sb, bv_d.ap())
            for i in range(n_pat):
                nc.sync.dma_start(mpat[i], mpat_d.ap()[i])

            # resident inputs: x (transposed), all weights
            xT = [persist.tile([128, S], F16, name=f"x{ch}", tag=f"x{ch}")
                  for ch in range(NCH)]
            for ch in range(NCH):
                nc.sync.dma_start(xT[ch], xT_c[ch])
            wqk_sb = [persist.tile([128, 2 * DC], F16, name=f"wqk{ch}",
                                   tag=f"wqk{ch}") for ch in range(NCH)]
            for ch in range(NCH):
                nc.sync.dma_start(wqk_sb[ch], wqk_c[ch])
            wv_sb = [persist.tile([128, DC], F16, name=f"wv{ch}",
                                  tag=f"wv{ch}") for ch in range(NCH)]
            for ch in range(NCH):
                nc.sync.dma_start(wv_sb[ch], wv_c[ch])
            wo_sb = [persist.tile([128, D], F16, name=f"wo{p}", tag=f"wo{p}")
                     for p in range(NPAIR)]
            for p in range(NPAIR):
                nc.sync.dma_start(wo_sb[p], wo_c[p])

            # ---- projection chain emitters ----------------------------
            def emit_qk_chain(p, tt, psA):
                """qkT[p][:, tt*QT:...] = (w_qk[:, p-slice].T @ x.T) + bias."""
                ps = psA.tile([128, QT], F32, tag="pa", name=f"psqk{p}_{tt}")
                for ch in range(NCH):
                    nc.tensor.matmul(
                        ps, wqk_sb[ch][:, p * 128:(p + 1) * 128],
                        xT[ch][:, tt * QT:(tt + 1) * QT],
                        start=(ch == 0), stop=False)
                nc.tensor.matmul(
                    ps, bqk_sb[:, p * 128:(p + 1) * 128], ones_row,
                    start=False, stop=True)
                nc.vector.tensor_copy(qkT[p][:, tt * QT:(tt + 1) * QT], ps)

            def emit_v_chain(tt, psA):
                """vext[tt] <- x[tt-tokens] @ w_v + bias, plus ones column."""
                ps = psA.tile([128, DC], F32, tag="pa", name=f"psv{tt}")
                for ch in range(NCH):
                    nc.tensor.matmul(
                        ps, xT[ch][:, tt * 128:(tt + 1) * 128], wv_sb[ch],
                        start=(ch == 0), stop=False)
                nc.tensor.matmul(
                    ps, ones_row[:, 0:128], bv_sb, start=False, stop=True)
                src3 = ps.rearrange("p (h d) -> p h d", h=HPC)
                nc.vector.tensor_copy(vext[tt][:, :, 0:HD], src3)
                nc.vector.memset(vext[tt][:, :, HD:HD + 1], 1.0)

            # ---- out-projection chain emitter -------------------------
            def emit_d_chain(tt, nt, pairs, dst, psD, dcopy):
                pso = psD.tile([128, QT], F32, name=f"pso{pairs[0]}_{tt}_{nt}",
                               tag="pa")
                for i, p in enumerate(pairs):
                    nc.tensor.matmul(
                        pso, outTn[p][:, tt * 128:(tt + 1) * 128],
                        wo_sb[p][:, nt * QT:(nt + 1) * QT],
                        start=(i == 0), stop=(i == len(pairs) - 1))
                ot = dcopy.tile([128, QT], F32, tag="oc")
                nc.vector.tensor_copy(ot, pso)
                nc.sync.dma_start(
                    dst.ap()[tt * 128:(tt + 1) * 128,
                             nt * QT:(nt + 1) * QT], ot)

            # ---- attention event emitter ------------------------------
            def pair_events(p):
                evs = []
                for qt in range(NQT):
                    kts = [kt for kt in range(NKT) if classes[kt, qt] != 0]
                    for i, kt in enumerate(kts):
                        evs.append((p, qt, kt, i == 0, i == len(kts) - 1))
                return evs

            oT_live = {}
            pAB_live = {}

            def emit_sexp(ev, psS, psO, ppool):
                p, qt, kt, first, last = ev
                qTp, kTp = qkT[p], qkT[NPAIR + p]
                if first:
                    oT_live[(p, qt)] = [
                        psO.tile([HD + 1, QT], F32,
                                 name=f"o{p}_{qt}_{h}", tag=f"o{p % 2}_{h}")
                        for h in range(2)]
                sAB = psS.tile([128, 2, QT], F32, tag="sAB",
                               name=f"s{p}_{qt}_{kt}")
                for h in range(2):
                    nc.tensor.matmul(
                        sAB[:, h, :],
                        kTp[64 * h:64 * h + 64, kt * KT:(kt + 1) * KT],
                        qTp[64 * h:64 * h + 64, qt * QT:(qt + 1) * QT],
                        tile_position=(64 * h, 0))
                pAB = ppool.tile([128, 2, QT], F16, tag="pAB",
                                 name=f"p{p}_{qt}_{kt}")
                zlo, mlo, mhi = (0, 0, 0) if classes[kt, qt] == 1 \
                    else bounds[(kt, qt)]
                if zlo:
                    nc.vector.memset(pAB[:, :, 0:zlo], 0.0)
                nc.scalar.activation(
                    pAB[:, :, zlo:QT], sAB[:, :, zlo:QT],
                    mybir.ActivationFunctionType.Exp, scale=SCALE)
                if mhi > mlo:
                    pat = mpat[pat_idx[(kt, qt)]]
                    for h in range(2):
                        nc.vector.tensor_mul(
                            pAB[:, h, mlo:mhi], pAB[:, h, mlo:mhi],
                            pat[:, mlo:mhi])
                pAB_live[(p, qt, kt)] = pAB

            def emit_av(ev, spool):
                p, qt, kt, first, last = ev
                oT = oT_live[(p, qt)]
                pAB = pAB_live.pop((p, qt, kt))
                for h in range(2):
                    nc.tensor.matmul(
                        oT[h], vext[kt][:, 2 * p + h, :], pAB[:, h, :],
                        start=first, stop=last)
                if last:
                    for h in range(2):
                        den = spool.tile([1, QT], F32, tag="den",
                                         name=f"d{p}_{qt}_{h}")
                        nc.vector.tensor_copy(den, oT[h][HD:HD + 1, :])
                        rec = spool.tile([1, QT], F32, tag="rec",
                                         name=f"r{p}_{qt}_{h}")
                        nc.vector.reciprocal_approx_fast(out=rec, in_=den)
                        bc = spool.tile([HD, QT], F32, tag="bc",
                                        name=f"b{p}_{qt}_{h}")
                        nc.gpsimd.partition_broadcast(bc, rec[0:1, :])
                        nc.vector.tensor_mul(
                            outTn[p][64 * h:64 * h + 64,
                                     qt * QT:(qt + 1) * QT],
                            oT[h][0:HD, :], bc)

            LAG = 3

            def emit_group(evs, psS, psO, ppool, spool):
                for i in range(len(evs) + LAG):
                    if i < len(evs):
                        emit_sexp(evs[i], psS, psO, ppool)
                    j = i - LAG
                    if j >= 0:
                        emit_av(evs[j], spool)

            def merged_events(pa, pb):
                ea, eb = pair_events(pa), pair_events(pb)
                out = []
                for i in range(max(len(ea), len(eb))):
                    if i < len(ea):
                        out.append(ea[i])
                    if i < len(eb):
                        out.append(eb[i])
                return out

            # ---- head: qk for pairs 0/1, then v chains ----------------
            with nc.named_scope("head"), \
                    tc.tile_pool(name="psA", bufs=3, space="PSUM") as psA:
                for p in QK01:
                    for tt in range(NQT):
                        emit_qk_chain(p, tt, psA)
                for tt in range(NTT):
                    emit_v_chain(tt, psA)

            # ---- attention (projection chains woven in) ---------------
            with (
                tc.tile_pool(name="ppool", bufs=6) as ppool,
                tc.tile_pool(name="spool", bufs=4) as spool,
            ):
                with (
                    tc.tile_pool(name="psS0", bufs=2, space="PSUM") as psS,
                    tc.tile_pool(name="psO0", bufs=1, space="PSUM") as psO,
                ):
                    with nc.named_scope("attn_g0"):
                        emit_group(merged_events(0, 1), psS, psO, ppool, spool)
                with nc.named_scope("qk23"), \
                        tc.tile_pool(name="psA2", bufs=3, space="PSUM") as psA2, \
                        tc.tile_pool(name="dcopy01", bufs=3) as dcopy01:
                    for p in QK23:
                        for tt in range(NQT):
                            emit_qk_chain(p, tt, psA2)
                    for tt in range(NTT):
                        for nt in range(2):
                            emit_d_chain(tt, nt, [0, 1], out01_d, psA2,
                                         dcopy01)
                with (
                    tc.tile_pool(name="psS1", bufs=2, space="PSUM") as psS,
                    tc.tile_pool(name="psO1", bufs=1, space="PSUM") as psO,
                ):
                    with nc.named_scope("attn_g1"):
                        emit_group(merged_events(2, 3), psS, psO, ppool, spool)

            # ---- stage D tail: pairs 2/3 partial out-projection -------
            with (
                tc.tile_pool(name="dcopy", bufs=4) as dcopy,
                tc.tile_pool(name="psD", bufs=4, space="PSUM") as psD,
            ):
                with nc.named_scope("sD"):
                    for tt in range(NTT):
                        for nt in range(2):
                            emit_d_chain(tt, nt, [2, 3], out23_d, psD, dcopy)

    nc.compile()
    return nc


def _prepare_inputs(x, mask, w_qkv, b_qkv, w_out):
    classes, patterns, pat_idx, bounds = _classify_mask(np.asarray(mask))
    in_maps = []
    for c in range(NCORES):
        b, g = c // 2, c % 2
        h0 = g * HPC
        xT = np.ascontiguousarray(x[b].T.astype(np.float16))
        wq = w_qkv[:, h0 * HD:h0 * HD + DC]
        wk = w_qkv[:, D + h0 * HD:D + h0 * HD + DC]
        wv = w_qkv[:, 2 * D + h0 * HD:2 * D + h0 * HD + DC]
        bq = b_qkv[h0 * HD:h0 * HD + DC]
        bk = b_qkv[D + h0 * HD:D + h0 * HD + DC]
        bv = b_qkv[2 * D + h0 * HD:2 * D + h0 * HD + DC]
        wo = w_out[h0 * HD:h0 * HD + DC, :]
        in_maps.append({
            "xT": xT,
            "wqk": np.ascontiguousarray(
                np.concatenate([wq, wk], axis=1).astype(np.float16)),
            "wv": np.ascontiguousarray(wv.astype(np.float16)),
            "wo": np.ascontiguousarray(wo.astype(np.float16)),
            "bqk": np.ascontiguousarray(
                np.concatenate([bq, bk])[None, :].astype(np.float16)),
            "bv": np.ascontiguousarray(bv[None, :].astype(np.float16)),
            "mpat": patterns.astype(np.float16),
        })
    return classes, patterns, pat_idx, bounds, in_maps


def run(x, mask, w_qkv, b_qkv, w_out, b_out, trace=False):
    classes, patterns, pat_idx, bounds, in_maps = _prepare_inputs(
        x, mask, w_qkv, b_qkv, w_out)
    key = (classes.tobytes(), patterns.tobytes())
    if key not in _cache:
        _cache[key] = _build(classes, pat_idx, bounds, patterns.shape[0])
    nc = _cache[key]
    res = bass_utils.run_bass_kernel_spmd(
        nc, in_maps, core_ids=list(range(NCORES)), trace=trace)
    out = np.empty((B, S, D), np.float32)
    bo = np.asarray(b_out, np.float32)
    for b in range(B):
        out[b] = (res.results[2 * b]["out01"] + res.results[2 * b]["out23"]
                  + res.results[2 * b + 1]["out01"]
                  + res.results[2 * b + 1]["out23"] + bo)
    return out, res


def kernel(x, mask, w_qkv, b_qkv, w_out, b_out):
    out, _ = run(x, mask, w_qkv, b_qkv, w_out, b_out, trace=False)
    return out


# revision 19
# speedup vs baseline: 1.1282x; 1.1282x over previous
"""Multi-head causal attention (B=4, S=2048, D=1024, H=16) on 8 TRN2 NeuronCores.

Sharding: batch x head-group. Core c handles batch c//2 and heads
8*(c%2) .. 8*(c%2)+8 (tensor parallel over heads). Each core computes its
8 heads' attention plus a partial output projection; the host sums the two
partials per batch and adds b_out.

Device pipeline (per core):
  - projections in fp16 (x, weights pre-cast on host), attention scores via
    row-packed (tile_position) fp16 matmuls in S^T [k, q] layout, exp on
    ScalarE straight out of PSUM, fp16 P with multiplicative mask tiles for
    diagonal blocks (block structure derived from the actual mask input;
    fully-masked blocks skipped, leading masked columns excluded from exp).
  - attn@V with lhsT = [v_h | ones] (M=65): row 64 accumulates softmax
    denominators; normalization = reciprocal_approx_fast + GpSimd partition
    broadcast + multiply.
  - projection chains for later head pairs are emitted interleaved into the
    attention event stream so the PE never drains while ScalarE runs exp.
  - partial out-projection [token, d_model] per core; host sums partials.
"""
import numpy as np

import concourse.bass as bass
import concourse.tile as tile
from concourse import bacc, mybir
from concourse import bass_utils

B, S, D, H, HD = 4, 2048, 1024, 16, 64
NCORES = 8
HPC = H // 2          # heads per core (8)
NPAIR = HPC // 2      # head pairs per core (4)
DC = HPC * HD         # attn dims per core (512)
QT = 512              # q tile (free dim of S^T)
KT = 128              # k tile (partition dim of S^T)
NQT = S // QT         # 4
NKT = S // KT         # 16
NTT = S // 128        # 16 token tiles
NCH = D // 128        # 8 d_model chunks
SCALE = HD ** -0.5

F32 = mybir.dt.float32
F32R = mybir.dt.float32r
F16 = mybir.dt.float16

_cache = {}


def _classify_mask(mask):
    """Per (kt, qt) block: 0=skip (all masked), 1=full (none masked), 2=partial."""
    classes = np.zeros((NKT, NQT), np.int8)
    patterns = []
    pat_idx = {}
    pat_key = {}
    bounds = {}
    for qt in range(NQT):
        mb = mask[qt * QT:(qt + 1) * QT, :]          # [512, S] (q, k)
        for kt in range(NKT):
            blk = mb[:, kt * KT:(kt + 1) * KT]       # [512, 128] (q, k)
            if blk.all():
                classes[kt, qt] = 0
            elif not blk.any():
                classes[kt, qt] = 1
            else:
                classes[kt, qt] = 2
                tilev = (~blk).T.astype(np.float32)  # [128, 512] (k, q), 1=keep
                col_masked = tilev.min(axis=0) == 0.0
                col_dead = tilev.max(axis=0) == 0.0
                zlo = 0
                while zlo < QT and col_dead[zlo]:
                    zlo += 1
                nz = np.nonzero(col_masked[zlo:])[0]
                if len(nz):
                    mlo, mhi = zlo + int(nz[0]), zlo + int(nz[-1]) + 1
                else:
                    mlo, mhi = zlo, zlo
                key = tilev.tobytes()
                if key not in pat_key:
                    pat_key[key] = len(patterns)
                    patterns.append(tilev)
                pat_idx[(kt, qt)] = pat_key[key]
                bounds[(kt, qt)] = (zlo, mlo, mhi)
    if not patterns:
        patterns.append(np.ones((KT, QT), np.float32))
    return classes, np.stack(patterns), pat_idx, bounds


def _build(classes, pat_idx, bounds, n_pat):
    nc = bacc.Bacc("TRN2", target_bir_lowering=False, debug=False,
                   num_devices=NCORES)

    xT_d = nc.dram_tensor("xT", [D, S], F16, kind="ExternalInput")
    wqk_d = nc.dram_tensor("wqk", [D, 2 * DC], F16, kind="ExternalInput")
    wv_d = nc.dram_tensor("wv", [D, DC], F16, kind="ExternalInput")
    wo_d = nc.dram_tensor("wo", [DC, D], F16, kind="ExternalInput")
    bqk_d = nc.dram_tensor("bqk", [1, 2 * DC], F16, kind="ExternalInput")
    bv_d = nc.dram_tensor("bv", [1, DC], F16, kind="ExternalInput")
    mpat_d = nc.dram_tensor("mpat", [n_pat, KT, QT], F16, kind="ExternalInput")
    out01_d = nc.dram_tensor("out01", [S, D], F32, kind="ExternalOutput")
    out23_d = nc.dram_tensor("out23", [S, D], F32, kind="ExternalOutput")

    wqk_c = wqk_d.ap().rearrange("(c p) n -> c p n", p=128)
    wv_c = wv_d.ap().rearrange("(c p) n -> c p n", p=128)
    wo_c = wo_d.ap().rearrange("(c p) n -> c p n", p=128)
    xT_c = xT_d.ap().rearrange("(c p) n -> c p n", p=128)

    # qk production order: pair tiles (q_p at index p, k_p at NPAIR+p)
    QK01 = [0, 4, 1, 5]          # q0 k0 q1 k1 — before attention group 0
    QK23 = [2, 6, 3, 7]          # produced inside attention group 0

    with tile.TileContext(nc) as tc:
        with (
            tc.tile_pool(name="persist", bufs=1) as persist,
            tc.tile_pool(name="bigpool", bufs=1) as bigpool,
        ):
            # ---- persistent tiles + all input DMAs --------------------
            qkT = [bigpool.tile([128, S], F16, name=f"qkT{p}", tag="qk",
                                bufs=8) for p in range(2 * NPAIR)]
            vext = [persist.tile([128, HPC, HD + 1], F16, name=f"vx{t}",
                                 tag=f"vx{t}") for t in range(NTT)]
            mpat = [persist.tile([KT, QT], F16, name=f"mp{i}", tag=f"mp{i}")
                    for i in range(n_pat)]
            outTn = [bigpool.tile([128, S], F16, name=f"oTn{p}", tag="big",
                                  bufs=4) for p in range(NPAIR)]
            ones_row = persist.tile([1, QT], F16)
            nc.vector.memset(ones_row, 1.0)
            bqk_sb = persist.tile([1, 2 * DC], F16)
            nc.sync.dma_start(bqk_sb, bqk_d.ap())
            bv_sb = persist.tile([1, DC], F16)
            nc.sync.dma_start(bv_sb, bv_d.ap())
            for i in range(n_pat):
                nc.sync.dma_start(mpat[i], mpat_d.ap()[i])

            # resident inputs: x (transposed), all weights
            xT = [persist.tile([128, S], F16, name=f"x{ch}", tag=f"x{ch}")
                  for ch in range(NCH)]
            for ch in range(NCH):
                nc.sync.dma_start(xT[ch], xT_c[ch])
            wqk_sb = [persist.tile([128, 2 * DC], F16, name=f"wqk{ch}",
                                   tag=f"wqk{ch}") for ch in range(NCH)]
            for ch in range(NCH):
                nc.sync.dma_start(wqk_sb[ch], wqk_c[ch])
            wv_sb = [persist.tile([128, DC], F16, name=f"wv{ch}",
                                  tag=f"wv{ch}") for ch in range(NCH)]
            for ch in range(NCH):
                nc.sync.dma_start(wv_sb[ch], wv_c[ch])
            wo_sb = [persist.tile([128, D], F16, name=f"wo{p}", tag=f"wo{p}")
                     for p in range(NPAIR)]
            for p in range(NPAIR):
                nc.sync.dma_start(wo_sb[p], wo_c[p])

            # ---- projection chain emitters ----------------------------
            def emit_qk_chain(p, tt, psA):
                """qkT[p][:, tt*QT:...] = (w_qk[:, p-slice].T @ x.T) + bias."""
                ps = psA.tile([128, QT], F32, tag="pa", name=f"psqk{p}_{tt}")
                for ch in range(NCH):
                    nc.tensor.matmul(
                        ps, wqk_sb[ch][:, p * 128:(p + 1) * 128],
                        xT[ch][:, tt * QT:(tt + 1) * QT],
                        start=(ch == 0), stop=False)
                nc.tensor.matmul(
                    ps, bqk_sb[:, p * 128:(p + 1) * 128], ones_row,
                    start=False, stop=True)
                nc.vector.tensor_copy(qkT[p][:, tt * QT:(tt + 1) * QT], ps)

            def emit_v_chain(tt, psA):
                """vext[tt] <- x[tt-tokens] @ w_v + bias, plus ones column."""
                ps = psA.tile([128, DC], F32, tag="pa", name=f"psv{tt}")
                for ch in range(NCH):
                    nc.tensor.matmul(
                        ps, xT[ch][:, tt * 128:(tt + 1) * 128], wv_sb[ch],
                        start=(ch == 0), stop=False)
                nc.tensor.matmul(
                    ps, ones_row[:, 0:128], bv_sb, start=False, stop=True)
                src3 = ps.rearrange("p (h d) -> p h d", h=HPC)
                nc.vector.tensor_copy(vext[tt][:, :, 0:HD], src3)
                nc.vector.memset(vext[tt][:, :, HD:HD + 1], 1.0)

            # ---- out-projection chain emitter -------------------------
            def emit_d_chain(tt, nt, pairs, dst, psD, dcopy):
                pso = psD.tile([128, QT], F32, name=f"pso{pairs[0]}_{tt}_{nt}",
                               tag="pa")
                for i, p in enumerate(pairs):
                    nc.tensor.matmul(
                        pso, outTn[p][:, tt * 128:(tt + 1) * 128],
                        wo_sb[p][:, nt * QT:(nt + 1) * QT],
                        start=(i == 0), stop=(i == len(pairs) - 1))
                ot = dcopy.tile([128, QT], F32, tag="oc")
                if (tt + nt) % 2 == 0:
                    nc.vector.tensor_copy(ot, pso)
                else:
                    nc.scalar.activation(
                        ot, pso, mybir.ActivationFunctionType.Copy)
                nc.sync.dma_start(
                    dst.ap()[tt * 128:(tt + 1) * 128,
                             nt * QT:(nt + 1) * QT], ot)

            # ---- attention event emitter ------------------------------
            def pair_events(p):
                evs = []
                for qt in range(NQT):
                    kts = [kt for kt in range(NKT) if classes[kt, qt] != 0]
                    for i, kt in enumerate(kts):
                        evs.append((p, qt, kt, i == 0, i == len(kts) - 1))
                return evs

            oT_live = {}
            pAB_live = {}

            def emit_sexp(ev, psS, psO, ppool):
                p, qt, kt, first, last = ev
                qTp, kTp = qkT[p], qkT[NPAIR + p]
                if first:
                    oT_live[(p, qt)] = [
                        psO.tile([HD + 1, QT], F32,
                                 name=f"o{p}_{qt}_{h}", tag=f"o{p % 2}_{h}")
                        for h in range(2)]
                sAB = psS.tile([128, 2, QT], F32, tag="sAB",
                               name=f"s{p}_{qt}_{kt}")
                for h in range(2):
                    nc.tensor.matmul(
                        sAB[:, h, :],
                        kTp[64 * h:64 * h + 64, kt * KT:(kt + 1) * KT],
                        qTp[64 * h:64 * h + 64, qt * QT:(qt + 1) * QT],
                        tile_position=(64 * h, 0))
                pAB = ppool.tile([128, 2, QT], F16, tag="pAB",
                                 name=f"p{p}_{qt}_{kt}")
                zlo, mlo, mhi = (0, 0, 0) if classes[kt, qt] == 1 \
                    else bounds[(kt, qt)]
                if zlo:
                    nc.vector.memset(pAB[:, :, 0:zlo], 0.0)
                nc.scalar.activation(
                    pAB[:, :, zlo:QT], sAB[:, :, zlo:QT],
                    mybir.ActivationFunctionType.Exp, scale=SCALE)
                if mhi > mlo:
                    pat = mpat[pat_idx[(kt, qt)]]
                    for h in range(2):
                        nc.vector.tensor_mul(
                            pAB[:, h, mlo:mhi], pAB[:, h, mlo:mhi],
                            pat[:, mlo:mhi])
                pAB_live[(p, qt, kt)] = pAB

            def emit_av(ev, spool):
                p, qt, kt, first, last = ev
                oT = oT_live[(p, qt)]
                pAB = pAB_live.pop((p, qt, kt))
                for h in range(2):
                    nc.tensor.matmul(
                        oT[h], vext[kt][:, 2 * p + h, :], pAB[:, h, :],
                        start=first, stop=last)
                if last:
                    for h in range(2):
                        den = spool.tile([1, QT], F32, tag="den",
                                         name=f"d{p}_{qt}_{h}")
                        nc.vector.tensor_copy(den, oT[h][HD:HD + 1, :])
                        rec = spool.tile([1, QT], F32, tag="rec",
                                         name=f"r{p}_{qt}_{h}")
                        nc.vector.reciprocal_approx_fast(out=rec, in_=den)
                        bc = spool.tile([HD, QT], F32, tag="bc",
                                        name=f"b{p}_{qt}_{h}")
                        nc.gpsimd.partition_broadcast(bc, rec[0:1, :])
                        nc.vector.tensor_mul(
                            outTn[p][64 * h:64 * h + 64,
                                     qt * QT:(qt + 1) * QT],
                            oT[h][0:HD, :], bc)

            LAG = 3

            def emit_group(evs, psS, psO, ppool, spool):
                for i in range(len(evs) + LAG):
                    if i < len(evs):
                        emit_sexp(evs[i], psS, psO, ppool)
                    j = i - LAG
                    if j >= 0:
                        emit_av(evs[j], spool)

            def merged_events(pa, pb):
                ea, eb = pair_events(pa), pair_events(pb)
                out = []
                for i in range(max(len(ea), len(eb))):
                    if i < len(ea):
                        out.append(ea[i])
                    if i < len(eb):
                        out.append(eb[i])
                return out

            # ---- head: qk for pairs 0/1, then v chains ----------------
            with nc.named_scope("head"), \
                    tc.tile_pool(name="psA", bufs=3, space="PSUM") as psA:
                for p in QK01:
                    for tt in range(NQT):
                        emit_qk_chain(p, tt, psA)
                for tt in range(NTT):
                    emit_v_chain(tt, psA)

            # ---- attention (projection chains woven in) ---------------
            with (
                tc.tile_pool(name="ppool", bufs=6) as ppool,
                tc.tile_pool(name="spool", bufs=4) as spool,
            ):
                with (
                    tc.tile_pool(name="psS0", bufs=2, space="PSUM") as psS,
                    tc.tile_pool(name="psO0", bufs=1, space="PSUM") as psO,
                ):
                    with nc.named_scope("attn_g0"):
                        emit_group(merged_events(0, 1), psS, psO, ppool, spool)
                with nc.named_scope("qk23"), \
                        tc.tile_pool(name="psA2", bufs=3, space="PSUM") as psA2:
                    for p in QK23:
                        for tt in range(NQT):
                            emit_qk_chain(p, tt, psA2)
                with (
                    tc.tile_pool(name="psS1", bufs=2, space="PSUM") as psS,
                    tc.tile_pool(name="psO1", bufs=1, space="PSUM") as psO,
                ):
                    with nc.named_scope("attn_g1"):
                        emit_group(merged_events(2, 3), psS, psO, ppool, spool)

            # ---- stage D: full out-projection -------------------------
            with (
                tc.tile_pool(name="dcopy", bufs=6) as dcopy,
                tc.tile_pool(name="psD", bufs=4, space="PSUM") as psD,
            ):
                with nc.named_scope("sD"):
                    for tt in range(NTT):
                        for nt in range(2):
                            emit_d_chain(tt, nt, [0, 1, 2, 3], out01_d,
                                         psD, dcopy)

    nc.compile()
    return nc


def _prepare_inputs(x, mask, w_qkv, b_qkv, w_out):
    classes, patterns, pat_idx, bounds = _classify_mask(np.asarray(mask))
    in_maps = []
    for c in range(NCORES):
        b, g = c // 2, c % 2
        h0 = g * HPC
        xT = np.ascontiguousarray(x[b].T.astype(np.float16))
        wq = w_qkv[:, h0 * HD:h0 * HD + DC]
        wk = w_qkv[:, D + h0 * HD:D + h0 * HD + DC]
        wv = w_qkv[:, 2 * D + h0 * HD:2 * D + h0 * HD + DC]
        bq = b_qkv[h0 * HD:h0 * HD + DC]
        bk = b_qkv[D + h0 * HD:D + h0 * HD + DC]
        bv = b_qkv[2 * D + h0 * HD:2 * D + h0 * HD + DC]
        wo = w_out[h0 * HD:h0 * HD + DC, :]
        in_maps.append({
            "xT": xT,
            "wqk": np.ascontiguousarray(
                np.concatenate([wq, wk], axis=1).astype(np.float16)),
            "wv": np.ascontiguousarray(wv.astype(np.float16)),
            "wo": np.ascontiguousarray(wo.astype(np.float16)),
            "bqk": np.ascontiguousarray(
                np.concatenate([bq, bk])[None, :].astype(np.float16)),
            "bv": np.ascontiguousarray(bv[None, :].astype(np.float16)),
            "mpat": patterns.astype(np.float16),
        })
    return classes, patterns, pat_idx, bounds, in_maps


def run(x, mask, w_qkv, b_qkv, w_out, b_out, trace=False):
    classes, patterns, pat_idx, bounds, in_maps = _prepare_inputs(
        x, mask, w_qkv, b_qkv, w_out)
    key = (classes.tobytes(), patterns.tobytes())
    if key not in _cache:
        _cache[key] = _build(classes, pat_idx, bounds, patterns.shape[0])
    nc = _cache[key]
    res = bass_utils.run_bass_kernel_spmd(
        nc, in_maps, core_ids=list(range(NCORES)), trace=trace)
    out = np.empty((B, S, D), np.float32)
    bo = np.asarray(b_out, np.float32)
    for b in range(B):
        out[b] = (res.results[2 * b]["out01"]
                  + res.results[2 * b + 1]["out01"] + bo)
    return out, res


def kernel(x, mask, w_qkv, b_qkv, w_out, b_out):
    out, _ = run(x, mask, w_qkv, b_qkv, w_out, b_out, trace=False)
    return out


# revision 20
# speedup vs baseline: 1.1837x; 1.0492x over previous
"""Multi-head causal attention (B=4, S=2048, D=1024, H=16) on 8 TRN2 NeuronCores.

Sharding: batch x head-group. Core c handles batch c//2 and heads
8*(c%2) .. 8*(c%2)+8 (tensor parallel over heads). Each core computes its
8 heads' attention plus a partial output projection; the host sums the two
partials per batch and adds b_out.

Device pipeline (per core):
  - projections in fp16 (x, weights pre-cast on host), attention scores via
    row-packed (tile_position) fp16 matmuls in S^T [k, q] layout, exp on
    ScalarE straight out of PSUM, fp16 P with multiplicative mask tiles for
    diagonal blocks (block structure derived from the actual mask input;
    fully-masked blocks skipped, leading masked columns excluded from exp).
  - attn@V with lhsT = [v_h | ones] (M=65): row 64 accumulates softmax
    denominators; normalization = reciprocal_approx_fast + GpSimd partition
    broadcast + multiply.
  - projection chains for later head pairs are emitted interleaved into the
    attention event stream so the PE never drains while ScalarE runs exp.
  - partial out-projection [token, d_model] per core; host sums partials.
"""
import numpy as np

import concourse.bass as bass
import concourse.tile as tile
from concourse import bacc, mybir
from concourse import bass_utils

B, S, D, H, HD = 4, 2048, 1024, 16, 64
NCORES = 8
HPC = H // 2          # heads per core (8)
NPAIR = HPC // 2      # head pairs per core (4)
DC = HPC * HD         # attn dims per core (512)
QT = 512              # q tile (free dim of S^T)
KT = 128              # k tile (partition dim of S^T)
NQT = S // QT         # 4
NKT = S // KT         # 16
NTT = S // 128        # 16 token tiles
NCH = D // 128        # 8 d_model chunks
SCALE = HD ** -0.5

F32 = mybir.dt.float32
F32R = mybir.dt.float32r
F16 = mybir.dt.float16

_cache = {}


def _classify_mask(mask):
    """Per (kt, qt) block: 0=skip (all masked), 1=full (none masked), 2=partial."""
    classes = np.zeros((NKT, NQT), np.int8)
    patterns = []
    pat_idx = {}
    pat_key = {}
    bounds = {}
    for qt in range(NQT):
        mb = mask[qt * QT:(qt + 1) * QT, :]          # [512, S] (q, k)
        for kt in range(NKT):
            blk = mb[:, kt * KT:(kt + 1) * KT]       # [512, 128] (q, k)
            if blk.all():
                classes[kt, qt] = 0
            elif not blk.any():
                classes[kt, qt] = 1
            else:
                classes[kt, qt] = 2
                tilev = (~blk).T.astype(np.float32)  # [128, 512] (k, q), 1=keep
                col_masked = tilev.min(axis=0) == 0.0
                col_dead = tilev.max(axis=0) == 0.0
                zlo = 0
                while zlo < QT and col_dead[zlo]:
                    zlo += 1
                nz = np.nonzero(col_masked[zlo:])[0]
                if len(nz):
                    mlo, mhi = zlo + int(nz[0]), zlo + int(nz[-1]) + 1
                else:
                    mlo, mhi = zlo, zlo
                key = tilev.tobytes()
                if key not in pat_key:
                    pat_key[key] = len(patterns)
                    patterns.append(tilev)
                pat_idx[(kt, qt)] = pat_key[key]
                bounds[(kt, qt)] = (zlo, mlo, mhi)
    if not patterns:
        patterns.append(np.ones((KT, QT), np.float32))
    return classes, np.stack(patterns), pat_idx, bounds


def _build(classes, pat_idx, bounds, n_pat):
    nc = bacc.Bacc("TRN2", target_bir_lowering=False, debug=False,
                   num_devices=NCORES)

    xT_d = nc.dram_tensor("xT", [D, S], F16, kind="ExternalInput")
    wqk_d = nc.dram_tensor("wqk", [D, 2 * DC], F16, kind="ExternalInput")
    wv_d = nc.dram_tensor("wv", [D, DC], F16, kind="ExternalInput")
    wo_d = nc.dram_tensor("wo", [DC, D], F16, kind="ExternalInput")
    bqk_d = nc.dram_tensor("bqk", [128, 2 * NPAIR], F32, kind="ExternalInput")
    bv_d = nc.dram_tensor("bv", [1, DC], F32, kind="ExternalInput")
    mpat_d = nc.dram_tensor("mpat", [n_pat, KT, QT], F16, kind="ExternalInput")
    out01_d = nc.dram_tensor("out01", [S, D], F32, kind="ExternalOutput")

    wqk_c = wqk_d.ap().rearrange("(c p) n -> c p n", p=128)
    wv_c = wv_d.ap().rearrange("(c p) n -> c p n", p=128)
    wo_c = wo_d.ap().rearrange("(c p) n -> c p n", p=128)
    xT_c = xT_d.ap().rearrange("(c p) n -> c p n", p=128)

    # qk production order: pair tiles (q_p at index p, k_p at NPAIR+p)
    QK01 = [0, 4, 1, 5]          # q0 k0 q1 k1 — before attention group 0
    QK23 = [2, 6, 3, 7]          # produced inside attention group 0

    with tile.TileContext(nc) as tc:
        with (
            tc.tile_pool(name="persist", bufs=1) as persist,
            tc.tile_pool(name="bigpool", bufs=1) as bigpool,
        ):
            # ---- persistent tiles + all input DMAs --------------------
            qkT = [bigpool.tile([128, S], F16, name=f"qkT{p}", tag="qk",
                                bufs=8) for p in range(2 * NPAIR)]
            vext = [persist.tile([128, HPC, HD + 1], F16, name=f"vx{t}",
                                 tag=f"vx{t}") for t in range(NTT)]
            mpat = [persist.tile([KT, QT], F16, name=f"mp{i}", tag=f"mp{i}")
                    for i in range(n_pat)]
            outTn = [bigpool.tile([128, S], F16, name=f"oTn{p}", tag="big",
                                  bufs=4) for p in range(NPAIR)]
            bqk_sb = persist.tile([128, 2 * NPAIR], F32)
            nc.sync.dma_start(bqk_sb, bqk_d.ap())
            bv_bc = persist.tile([128, DC], F32)
            nc.sync.dma_start(
                bv_bc,
                bass.AP(tensor=bv_d, offset=0, ap=[[0, 128], [1, DC]]))
            for i in range(n_pat):
                nc.sync.dma_start(mpat[i], mpat_d.ap()[i])

            # resident inputs: x (transposed), all weights
            xT = [persist.tile([128, S], F16, name=f"x{ch}", tag=f"x{ch}")
                  for ch in range(NCH)]
            for ch in range(NCH):
                nc.sync.dma_start(xT[ch], xT_c[ch])
            wqk_sb = [persist.tile([128, 2 * DC], F16, name=f"wqk{ch}",
                                   tag=f"wqk{ch}") for ch in range(NCH)]
            for ch in range(NCH):
                nc.sync.dma_start(wqk_sb[ch], wqk_c[ch])
            wv_sb = [persist.tile([128, DC], F16, name=f"wv{ch}",
                                  tag=f"wv{ch}") for ch in range(NCH)]
            for ch in range(NCH):
                nc.sync.dma_start(wv_sb[ch], wv_c[ch])
            wo_sb = [persist.tile([128, D], F16, name=f"wo{p}", tag=f"wo{p}")
                     for p in range(NPAIR)]
            for p in range(NPAIR):
                nc.sync.dma_start(wo_sb[p], wo_c[p])

            # ---- projection chain emitters ----------------------------
            def emit_qk_chain(p, tt, psA):
                """qkT[p][:, tt*QT:...] = (w_qk[:, p-slice].T @ x.T) + bias."""
                ps = psA.tile([128, QT], F32, tag="pa", name=f"psqk{p}_{tt}")
                for ch in range(NCH):
                    nc.tensor.matmul(
                        ps, wqk_sb[ch][:, p * 128:(p + 1) * 128],
                        xT[ch][:, tt * QT:(tt + 1) * QT],
                        start=(ch == 0), stop=(ch == NCH - 1))
                nc.vector.tensor_scalar_add(
                    qkT[p][:, tt * QT:(tt + 1) * QT], ps, bqk_sb[:, p:p + 1])

            def emit_v_chain(tt, psA):
                """vext[tt] <- x[tt-tokens] @ w_v + bias, plus ones column."""
                ps = psA.tile([128, DC], F32, tag="pa", name=f"psv{tt}")
                for ch in range(NCH):
                    nc.tensor.matmul(
                        ps, xT[ch][:, tt * 128:(tt + 1) * 128], wv_sb[ch],
                        start=(ch == 0), stop=(ch == NCH - 1))
                src3 = ps.rearrange("p (h d) -> p h d", h=HPC)
                bv3 = bv_bc.rearrange("p (h d) -> p h d", h=HPC)
                nc.vector.tensor_add(vext[tt][:, :, 0:HD], src3, bv3)
                nc.vector.memset(vext[tt][:, :, HD:HD + 1], 1.0)

            # ---- out-projection chain emitter -------------------------
            def emit_d_chain(tt, nt, pairs, dst, psD, dcopy):
                pso = psD.tile([128, QT], F32, name=f"pso{pairs[0]}_{tt}_{nt}",
                               tag="pa")
                for i, p in enumerate(pairs):
                    nc.tensor.matmul(
                        pso, outTn[p][:, tt * 128:(tt + 1) * 128],
                        wo_sb[p][:, nt * QT:(nt + 1) * QT],
                        start=(i == 0), stop=(i == len(pairs) - 1))
                ot = dcopy.tile([128, QT], F32, tag="oc")
                if (tt + nt) % 2 == 0:
                    nc.vector.tensor_copy(ot, pso)
                else:
                    nc.scalar.activation(
                        ot, pso, mybir.ActivationFunctionType.Copy)
                nc.sync.dma_start(
                    dst.ap()[tt * 128:(tt + 1) * 128,
                             nt * QT:(nt + 1) * QT], ot)

            # ---- attention event emitter ------------------------------
            def pair_events(p):
                evs = []
                for qt in range(NQT):
                    kts = [kt for kt in range(NKT) if classes[kt, qt] != 0]
                    for i, kt in enumerate(kts):
                        evs.append((p, qt, kt, i == 0, i == len(kts) - 1))
                return evs

            oT_live = {}
            pAB_live = {}

            def emit_sexp(ev, psS, psO, ppool):
                p, qt, kt, first, last = ev
                qTp, kTp = qkT[p], qkT[NPAIR + p]
                if first:
                    oT_live[(p, qt)] = [
                        psO.tile([HD + 1, QT], F32,
                                 name=f"o{p}_{qt}_{h}", tag=f"o{p % 2}_{h}")
                        for h in range(2)]
                sAB = psS.tile([128, 2, QT], F32, tag="sAB",
                               name=f"s{p}_{qt}_{kt}")
                for h in range(2):
                    nc.tensor.matmul(
                        sAB[:, h, :],
                        kTp[64 * h:64 * h + 64, kt * KT:(kt + 1) * KT],
                        qTp[64 * h:64 * h + 64, qt * QT:(qt + 1) * QT],
                        tile_position=(64 * h, 0))
                pAB = ppool.tile([128, 2, QT], F16, tag="pAB",
                                 name=f"p{p}_{qt}_{kt}")
                zlo, mlo, mhi = (0, 0, 0) if classes[kt, qt] == 1 \
                    else bounds[(kt, qt)]
                if zlo:
                    nc.vector.memset(pAB[:, :, 0:zlo], 0.0)
                nc.scalar.activation(
                    pAB[:, :, zlo:QT], sAB[:, :, zlo:QT],
                    mybir.ActivationFunctionType.Exp, scale=SCALE)
                if mhi > mlo:
                    pat = mpat[pat_idx[(kt, qt)]]
                    for h in range(2):
                        nc.vector.tensor_mul(
                            pAB[:, h, mlo:mhi], pAB[:, h, mlo:mhi],
                            pat[:, mlo:mhi])
                pAB_live[(p, qt, kt)] = pAB

            def emit_av(ev, spool):
                p, qt, kt, first, last = ev
                oT = oT_live[(p, qt)]
                pAB = pAB_live.pop((p, qt, kt))
                for h in range(2):
                    nc.tensor.matmul(
                        oT[h], vext[kt][:, 2 * p + h, :], pAB[:, h, :],
                        start=first, stop=last)
                if last:
                    for h in range(2):
                        den = spool.tile([1, QT], F32, tag="den",
                                         name=f"d{p}_{qt}_{h}")
                        nc.vector.tensor_copy(den, oT[h][HD:HD + 1, :])
                        rec = spool.tile([1, QT], F32, tag="rec",
                                         name=f"r{p}_{qt}_{h}")
                        nc.vector.reciprocal_approx_fast(out=rec, in_=den)
                        bc = spool.tile([HD, QT], F32, tag="bc",
                                        name=f"b{p}_{qt}_{h}")
                        nc.gpsimd.partition_broadcast(bc, rec[0:1, :])
                        nc.vector.tensor_mul(
                            outTn[p][64 * h:64 * h + 64,
                                     qt * QT:(qt + 1) * QT],
                            oT[h][0:HD, :], bc)

            LAG = 3

            def emit_group(evs, psS, psO, ppool, spool):
                for i in range(len(evs) + LAG):
                    if i < len(evs):
                        emit_sexp(evs[i], psS, psO, ppool)
                    j = i - LAG
                    if j >= 0:
                        emit_av(evs[j], spool)

            def merged_events(pa, pb):
                ea, eb = pair_events(pa), pair_events(pb)
                out = []
                for i in range(max(len(ea), len(eb))):
                    if i < len(ea):
                        out.append(ea[i])
                    if i < len(eb):
                        out.append(eb[i])
                return out

            # ---- head: qk for pairs 0/1, then v chains ----------------
            with nc.named_scope("head"), \
                    tc.tile_pool(name="psA", bufs=3, space="PSUM") as psA:
                for p in QK01:
                    for tt in range(NQT):
                        emit_qk_chain(p, tt, psA)
                for tt in range(NTT):
                    emit_v_chain(tt, psA)

            # ---- attention (projection chains woven in) ---------------
            with (
                tc.tile_pool(name="ppool", bufs=6) as ppool,
                tc.tile_pool(name="spool", bufs=4) as spool,
            ):
                with (
                    tc.tile_pool(name="psS0", bufs=2, space="PSUM") as psS,
                    tc.tile_pool(name="psO0", bufs=1, space="PSUM") as psO,
                ):
                    with nc.named_scope("attn_g0"):
                        emit_group(merged_events(0, 1), psS, psO, ppool, spool)
                with nc.named_scope("qk23"), \
                        tc.tile_pool(name="psA2", bufs=3, space="PSUM") as psA2:
                    for p in QK23:
                        for tt in range(NQT):
                            emit_qk_chain(p, tt, psA2)
                with (
                    tc.tile_pool(name="psS1", bufs=2, space="PSUM") as psS,
                    tc.tile_pool(name="psO1", bufs=1, space="PSUM") as psO,
                ):
                    with nc.named_scope("attn_g1"):
                        emit_group(merged_events(2, 3), psS, psO, ppool, spool)

            # ---- stage D: full out-projection -------------------------
            with (
                tc.tile_pool(name="dcopy", bufs=6) as dcopy,
                tc.tile_pool(name="psD", bufs=4, space="PSUM") as psD,
            ):
                with nc.named_scope("sD"):
                    for tt in range(NTT):
                        for nt in range(2):
                            emit_d_chain(tt, nt, [0, 1, 2, 3], out01_d,
                                         psD, dcopy)

    nc.compile()
    return nc


def _prepare_inputs(x, mask, w_qkv, b_qkv, w_out):
    classes, patterns, pat_idx, bounds = _classify_mask(np.asarray(mask))
    in_maps = []
    for c in range(NCORES):
        b, g = c // 2, c % 2
        h0 = g * HPC
        xT = np.ascontiguousarray(x[b].T.astype(np.float16))
        wq = w_qkv[:, h0 * HD:h0 * HD + DC]
        wk = w_qkv[:, D + h0 * HD:D + h0 * HD + DC]
        wv = w_qkv[:, 2 * D + h0 * HD:2 * D + h0 * HD + DC]
        bq = b_qkv[h0 * HD:h0 * HD + DC]
        bk = b_qkv[D + h0 * HD:D + h0 * HD + DC]
        bv = b_qkv[2 * D + h0 * HD:2 * D + h0 * HD + DC]
        wo = w_out[h0 * HD:h0 * HD + DC, :]
        in_maps.append({
            "xT": xT,
            "wqk": np.ascontiguousarray(
                np.concatenate([wq, wk], axis=1).astype(np.float16)),
            "wv": np.ascontiguousarray(wv.astype(np.float16)),
            "wo": np.ascontiguousarray(wo.astype(np.float16)),
            "bqk": np.ascontiguousarray(
                np.concatenate([bq, bk]).reshape(2 * NPAIR, 128).T
                .astype(np.float32)),
            "bv": np.ascontiguousarray(bv[None, :].astype(np.float32)),
            "mpat": patterns.astype(np.float16),
        })
    return classes, patterns, pat_idx, bounds, in_maps


def run(x, mask, w_qkv, b_qkv, w_out, b_out, trace=False):
    classes, patterns, pat_idx, bounds, in_maps = _prepare_inputs(
        x, mask, w_qkv, b_qkv, w_out)
    key = (classes.tobytes(), patterns.tobytes())
    if key not in _cache:
        _cache[key] = _build(classes, pat_idx, bounds, patterns.shape[0])
    nc = _cache[key]
    res = bass_utils.run_bass_kernel_spmd(
        nc, in_maps, core_ids=list(range(NCORES)), trace=trace)
    out = np.empty((B, S, D), np.float32)
    bo = np.asarray(b_out, np.float32)
    for b in range(B):
        out[b] = (res.results[2 * b]["out01"]
                  + res.results[2 * b + 1]["out01"] + bo)
    return out, res


def kernel(x, mask, w_qkv, b_qkv, w_out, b_out):
    out, _ = run(x, mask, w_qkv, b_qkv, w_out, b_out, trace=False)
    return out
